# revision 1
# baseline (speedup 1.0000x reference)
"""3-layer LSTM (B=256, T=512, I=128, H=64) + final linear, on 8 TRN2 NeuronCores.

Strategy:
  - Data-parallel: batch 256 -> 32 per core; weights replicated.
  - Per core, the 3 LSTM layers advance as a wavefront: at step s, layer l
    computes timestep t = s - l.  All layers' gates are packed into shared
    PSUM banks (bank A = [i;f] gate halves, bank B = [g;o]) so the
    activations/elementwise work runs as wide (3-layer) instructions.
  - Feature-major state layout [H, batch] so the recurrent h feeds the next
    matmul directly as the moving operand (no transposes on the critical
    path).  h is stored doubled (H2 = 2h) and the o-gate uses
    sigmoid(x) = (tanh(x/2)+1)/2 so that H2 = tanh(pre_o/2)*tanh(c) + tanh(c)
    costs two cheap vector ops; weight columns that consume h are pre-scaled
    by 0.5 on the host to compensate.
  - Per-layer biases enter via a K=3 "indicator" matmul that writes all three
    layers' bias columns into the PSUM bank in one instruction.
  - x is transposed/cast on the host to [I, T*Bc] bf16 and streamed.
"""
import numpy as np
import ml_dtypes

B, T, I, H = 256, 512, 128, 64
NCORES = 8
BC = B // NCORES            # 32 batch per core
NB = 3 * BC                 # 96: packed free width (3 layers x 32 batch)
XCHUNK = 16                 # timesteps per x DMA tile

BF16 = ml_dtypes.bfloat16
_cache = {}


def _prep_weights(inputs):
    f32 = np.float32
    # PyTorch gate row order: i(0:64) f(64:128) g(128:192) o(192:256).
    # Device layout: bank A rows = [f; i], bank B rows = [o; g] so that every
    # DVE tensor_tensor pairs operands at equal base partitions:
    #   i*g (base 64), f*c (base 0), o'*tanh(c) (base 0).
    permA = np.r_[64:128, 0:64]       # [f, i]
    permB = np.r_[192:256, 128:192]   # [o, g]
    W = {}
    for l in range(3):
        Wih = inputs[f'W_ih{l}'].astype(f32)
        Whh = inputs[f'W_hh{l}'].astype(f32)
        b = (inputs[f'b_ih{l}'] + inputs[f'b_hh{l}']).astype(f32)
        if l == 0:
            wxA = Wih[permA].T.copy()
            wxB = Wih[permB].T.copy()
            wxB[:, 0:64] *= 0.5                        # o-gate pre-scale
            W['wxA'], W['wxB'] = wxA.astype(BF16), wxB.astype(BF16)
            for perm, name in ((permA, 'wh0A'), (permB, 'wh0B')):
                m = np.zeros((128, 128), f32)
                m[64:128, :] = Whh[perm].T * 0.5       # h columns scaled (H2=2h)
                if name == 'wh0B':
                    m[:, 0:64] *= 0.5
                W[name] = m.astype(BF16)
        else:
            for perm, name in ((permA, f'w{l}A'), (permB, f'w{l}B')):
                m = np.concatenate([Wih[perm].T * 0.5, Whh[perm].T * 0.5], axis=0)
                if name.endswith('B'):
                    m[:, 0:64] *= 0.5
                W[name] = m.astype(BF16)
        bA = b[permA].copy()
        bB = b[permB].copy()
        bB[0:64] *= 0.5
        W.setdefault('biasA_rows', []).append(bA)
        W.setdefault('biasB_rows', []).append(bB)
    W['biasA'] = np.stack(W.pop('biasA_rows')).astype(f32)   # [3, 128]
    W['biasB'] = np.stack(W.pop('biasB_rows')).astype(f32)
    ind = np.zeros((3, NB), f32)
    for l in range(3):
        ind[l, 32 * l:32 * l + 32] = 1.0
    W['ind'] = ind
    W['wout'] = (inputs['W_out'].astype(f32).T * 0.5).astype(BF16)  # [64, 2]
    return W


def _build_program():
    import concourse.bass as bass
    import concourse.bacc as bacc
    import concourse.tile as tile
    from concourse import mybir

    AF = mybir.ActivationFunctionType
    bf16 = mybir.dt.bfloat16
    f32 = mybir.dt.float32

    nc = bacc.Bacc(None, target_bir_lowering=False, debug=False)
    xT_d = nc.dram_tensor("xT", [128, T * BC], bf16, kind="ExternalInput")
    wnames = ['wxA', 'wxB', 'wh0A', 'wh0B', 'w1A', 'w1B', 'w2A', 'w2B']
    wall_d = nc.dram_tensor("wall", [128, 8 * 128 + 2], bf16, kind="ExternalInput")
    fall_d = nc.dram_tensor("fall", [3, 256 + NB], f32, kind="ExternalInput")
    out_d = nc.dram_tensor("out", [2, BC], f32, kind="ExternalOutput")

    with tile.TileContext(nc) as tc:
        with (
            tc.tile_pool(name="singles", bufs=1) as singles,
            tc.tile_pool(name="xpool", bufs=3) as xpool,
            tc.tile_pool(name="scr", bufs=3) as scr,
            tc.tile_pool(name="psum", bufs=2, space="PSUM") as psum,
            tc.tile_pool(name="psum_o", bufs=1, space="PSUM") as psum_o,
        ):
            wall = singles.tile([128, 8 * 128 + 2], bf16, tag="wall")
            nc.sync.dma_start(out=wall, in_=wall_d[:, :])
            fall = singles.tile([3, 256 + NB], f32, tag="fall")
            nc.sync.dma_start(out=fall, in_=fall_d[:, :])
            ws = {n: wall[:, 128 * k:128 * (k + 1)] for k, n in enumerate(wnames)}
            wout = wall[0:64, 8 * 128:8 * 128 + 2]
            biasA = fall[:, 0:128]
            biasB = fall[:, 128:256]
            ind = fall[:, 256:256 + NB]

            V = singles.tile([128, NB], bf16, tag="V")     # [ [H2_{l-1}] ; [H2_l] ] per 32-col chunk
            C = singles.tile([64, NB], f32, tag="C")
            nc.vector.memset(V, 0.0)
            nc.vector.memset(C, 0.0)

            wA = {1: ws['w1A'], 2: ws['w2A']}
            wB = {1: ws['w1B'], 2: ws['w2B']}

            xtile = None
            H2_last = None
            for s in range(T + 2):
                ls = [l for l in (0, 1, 2) if 0 <= s - l < T]
                c0, c1 = min(ls) * 32, (max(ls) + 1) * 32
                cs = slice(c0, c1)

                if s % XCHUNK == 0 and s < T:
                    nch = min(XCHUNK, T - s)
                    xtile = xpool.tile([128, XCHUNK * BC], bf16, tag="xt")
                    nc.sync.dma_start(
                        out=xtile[:, 0:nch * BC], in_=xT_d[:, s * BC:(s + nch) * BC])

                pA = psum.tile([128, NB], f32, tag="pA")
                pB = psum.tile([128, NB], f32, tag="pB")
                # bias fill (start=True clears the accumulation window)
                nc.tensor.matmul(pA[:, cs], biasA, ind[:, cs],
                                 start=True, stop=False, skip_group_check=True)
                nc.tensor.matmul(pB[:, cs], biasB, ind[:, cs],
                                 start=True, stop=False, skip_group_check=True)
                if 0 in ls:
                    k = (s % XCHUNK) * BC
                    xs = xtile[:, k:k + BC]
                    nc.tensor.matmul(pA[:, 0:32], ws['wxA'], xs,
                                     start=False, stop=False, skip_group_check=True)
                    nc.tensor.matmul(pB[:, 0:32], ws['wxB'], xs,
                                     start=False, stop=False, skip_group_check=True)
                    nc.tensor.matmul(pA[:, 0:32], ws['wh0A'], V[:, 0:32],
                                     start=False, stop=True, skip_group_check=True)
                    nc.tensor.matmul(pB[:, 0:32], ws['wh0B'], V[:, 0:32],
                                     start=False, stop=True, skip_group_check=True)
                for l in (1, 2):
                    if l in ls:
                        cl = slice(32 * l, 32 * l + 32)
                        nc.tensor.matmul(pA[:, cl], wA[l], V[:, cl],
                                         start=False, stop=True, skip_group_check=True)
                        nc.tensor.matmul(pB[:, cl], wB[l], V[:, cl],
                                         start=False, stop=True, skip_group_check=True)

                Sif = scr.tile([128, NB], bf16, tag="Sif")
                Sgo = scr.tile([128, NB], bf16, tag="Sgo")
                Tc = scr.tile([64, NB], bf16, tag="Tc")
                U = scr.tile([64, NB], bf16, tag="U")
                H2 = scr.tile([64, NB], bf16, tag="H2")
                Pt = scr.tile([64, NB], f32, tag="Pt")
                Qt = scr.tile([64, NB], f32, tag="Qt")

                # bank A = [f; i] (sigmoid), bank B = [o; g] (tanh; o pre-halved)
                nc.scalar.activation(Sif[:, cs], pA[:, cs], AF.Sigmoid)
                nc.scalar.activation(Sgo[:, cs], pB[:, cs], AF.Tanh)
                nc.vector.tensor_mul(Pt[:, cs], Sif[64:128, cs], Sgo[64:128, cs])  # i*g
                nc.vector.tensor_mul(Qt[:, cs], Sif[0:64, cs], C[:, cs])           # f*c
                nc.vector.tensor_add(C[:, cs], Pt[:, cs], Qt[:, cs])
                nc.scalar.activation(Tc[:, cs], C[:, cs], AF.Tanh)
                nc.vector.tensor_mul(U[:, cs], Sgo[0:64, cs], Tc[:, cs])           # o'*tanh(c)
                nc.vector.tensor_add(H2[:, cs], U[:, cs], Tc[:, cs])               # H2 = 2h

                for l in ls:
                    cl = slice(32 * l, 32 * l + 32)
                    nc.vector.tensor_copy(V[64:128, cl], H2[:, cl])
                    if l < 2:
                        cn = slice(32 * (l + 1), 32 * (l + 1) + 32)
                        nc.vector.tensor_copy(V[0:64, cn], H2[:, cl])
                if s == T + 1:
                    H2_last = H2

            # final linear on h2(T-1):  out.T [2, BC] = (0.5*W_out).T.T @ H2
            H2f = singles.tile([64, BC], bf16, tag="H2f")
            nc.vector.tensor_copy(H2f, H2_last[:, 64:96])
            po = psum_o.tile([2, BC], f32, tag="po")
            nc.tensor.matmul(po, wout, H2f, start=True, stop=True)
            outT = singles.tile([2, BC], f32, tag="outT")
            nc.scalar.copy(outT, po)
            nc.sync.dma_start(out=out_d[:, :], in_=outT)

    nc.compile()
    return nc


def pack_operands(W):
    wall = np.zeros((128, 8 * 128 + 2), BF16)
    for k, n in enumerate(['wxA', 'wxB', 'wh0A', 'wh0B', 'w1A', 'w1B', 'w2A', 'w2B']):
        wall[:, 128 * k:128 * (k + 1)] = W[n]
    wall[0:64, 1024:1026] = W['wout']
    fall = np.zeros((3, 256 + NB), np.float32)
    fall[:, 0:128] = W['biasA']
    fall[:, 128:256] = W['biasB']
    fall[:, 256:256 + NB] = W['ind']
    return wall, fall


def make_in_maps(inputs):
    W = _prep_weights(inputs)
    wall, fall = pack_operands(W)
    x = inputs['x'].astype(np.float32)
    in_maps = []
    for c in range(NCORES):
        xc = x[c * BC:(c + 1) * BC]                        # [BC, T, I]
        xT = np.ascontiguousarray(xc.transpose(2, 1, 0).reshape(I, T * BC)).astype(BF16)
        in_maps.append({'xT': xT, 'wall': wall, 'fall': fall})
    return in_maps


def kernel(**inputs):
    from concourse.bass_utils import run_bass_kernel_spmd

    if 'nc' not in _cache:
        _cache['nc'] = _build_program()
    nc = _cache['nc']

    in_maps = make_in_maps(inputs)
    res = run_bass_kernel_spmd(nc, in_maps, list(range(NCORES)))
    outs = [res.results[c]['out'].T for c in range(NCORES)]   # each [BC, 2]
    full = np.concatenate(outs, axis=0).astype(np.float32)
    full = full + inputs['b_out'].astype(np.float32)[None, :]
    return full



# revision 2
# speedup vs baseline: 58.0268x; 58.0268x over previous
"""3-layer LSTM (B=256, T=512, I=128, H=64) + final linear, on 8 TRN2 NeuronCores.

Strategy (data-parallel: batch 256 -> 32 per core; weights replicated):
  - Wavefront over the 3 layers: at step s, layer l computes timestep
    t = s - l, so each step advances all three layers with one set of
    wide fused instructions and the serial dependency chain is T+2 steps.
  - ALL-TANH gates: the i,f,o rows of weights/biases are pre-scaled by
    0.5 so sigmoid(x) = (tanh(x/2)+1)/2; g rows stay full scale.  One
    fused activation instruction computes tanh over the whole [128,192]
    PSUM gate tile = all 4 gates x 3 layers (partition layout: [f;i]
    for cols 0:96, [o;g] for cols 96:192), replacing two per-bank
    activations.
  - Cell update via fused scalar_tensor_tensor (DVE-only instruction):
        P  = (TI + 1) * TG          # = 2*i*g
        Q  = (TF + 1) * C2          # = 4*f*c   (state C2 = 2c)
        C2 = (Q * 0.5) + P          # = 2c_new
    then TC = tanh(C2 * 0.5) on the ACT engine (scale folds the /2),
    and the h update is fused directly into the V-state writes:
        V[64:128, :] = (TO + 1) * TC      # = 2h   (state half)
        V[0:64, 32:96] = (TO + 1) * TC    # forward feed to layer l+1
    so no separate U/H2 tiles or tensor copies exist on the chain.
  - One bf16 bias matmul (K=6 indicator) initializes all 192 psum
    columns per step (start=True), replacing two fp32 bias matmuls.
  - V [128, 96] bf16: partitions 0:64 hold the input half (H2_{l-1}),
    64:128 the state half (H2_l); recurrent matmuls are K=128 with
    M=128 bf16 stationaries (FWL-eligible), 6 per step.
  - x is transposed/cast on the host to [I, T*Bc] bf16 and streamed in
    16-step chunks (triple buffered, off the critical chain).
"""
import numpy as np
import ml_dtypes

B, T, I, H = 256, 512, 128, 64
NCORES = 8
BC = B // NCORES            # 32 batch per core
NB = 3 * BC                 # 96
XCHUNK = 16

BF16 = ml_dtypes.bfloat16
_cache = {}

# PyTorch gate row order: i(0:64) f(64:128) g(128:192) o(192:256).
_permA = np.r_[64:128, 0:64]       # [f; i]
_permB = np.r_[192:256, 128:192]   # [o; g]
_sA = np.full(128, 0.5, np.float32)              # f,i rows halved
_sB = np.r_[np.full(64, 0.5, np.float32),        # o rows halved
            np.full(64, 1.0, np.float32)]        # g rows full


def _prep_weights(inputs):
    f32 = np.float32
    W = {}
    for l in range(3):
        Wih = inputs[f'W_ih{l}'].astype(f32)
        Whh = inputs[f'W_hh{l}'].astype(f32)
        b = (inputs[f'b_ih{l}'] + inputs[f'b_hh{l}']).astype(f32)
        for perm, s, tag in ((_permA, _sA, 'A'), (_permB, _sB, 'B')):
            # lhsT[k, gate] layout, gate row scale s, h-consuming cols x0.5
            if l == 0:
                W[f'wx{tag}'] = (Wih[perm].T * s[None, :]).astype(BF16)
                m = np.zeros((128, 128), f32)
                m[64:128, :] = Whh[perm].T * s[None, :] * 0.5
                W[f'w0{tag}'] = m.astype(BF16)
            else:
                m = np.concatenate([Wih[perm].T, Whh[perm].T], axis=0)
                m = m * s[None, :] * 0.5
                W[f'w{l}{tag}'] = m.astype(BF16)
            W.setdefault(f'bias{tag}', []).append(b[perm] * s)
    # bvals [6, 128]: rows 0..2 = A-region bias per layer, 3..5 = B-region.
    W['bvals'] = np.stack(W.pop('biasA') + W.pop('biasB')).astype(BF16)
    ind = np.zeros((6, 192), f32)
    for l in range(3):
        ind[l, 32 * l:32 * l + 32] = 1.0
        ind[3 + l, 96 + 32 * l:96 + 32 * l + 32] = 1.0
    W['ind'] = ind.astype(BF16)
    W['wout'] = (inputs['W_out'].astype(f32).T * 0.5).astype(BF16)  # [64, 2]
    return W


def _build_program():
    import concourse.bacc as bacc
    import concourse.tile as tile
    from concourse import mybir

    AF = mybir.ActivationFunctionType
    ALU = mybir.AluOpType
    bf16 = mybir.dt.bfloat16
    f32 = mybir.dt.float32

    nc = bacc.Bacc(None, target_bir_lowering=False, debug=False)
    xT_d = nc.dram_tensor("xT", [128, T * BC], bf16, kind="ExternalInput")
    wnames = ['wxA', 'wxB', 'w0A', 'w0B', 'w1A', 'w1B', 'w2A', 'w2B']
    wall_d = nc.dram_tensor("wall", [128, 8 * 128 + 2], bf16, kind="ExternalInput")
    fall_d = nc.dram_tensor("fall", [6, 128 + 192], bf16, kind="ExternalInput")
    out_d = nc.dram_tensor("out", [2, BC], f32, kind="ExternalOutput")

    with tile.TileContext(nc) as tc:
        with (
            tc.tile_pool(name="singles", bufs=1) as singles,
            tc.tile_pool(name="xpool", bufs=3) as xpool,
            tc.tile_pool(name="scr", bufs=3) as scr,
            tc.tile_pool(name="psum", bufs=2, space="PSUM") as psum,
            tc.tile_pool(name="psum_o", bufs=1, space="PSUM") as psum_o,
        ):
            wall = singles.tile([128, 8 * 128 + 2], bf16, tag="wall")
            nc.sync.dma_start(out=wall, in_=wall_d[:, :])
            fall = singles.tile([6, 128 + 192], bf16, tag="fall")
            nc.sync.dma_start(out=fall, in_=fall_d[:, :])
            ws = {n: wall[:, 128 * k:128 * (k + 1)] for k, n in enumerate(wnames)}
            wout = wall[0:64, 8 * 128:8 * 128 + 2]
            bvals = fall[:, 0:128]
            ind = fall[:, 128:128 + 192]

            V = singles.tile([128, NB], bf16, tag="V")
            C2 = singles.tile([64, NB], f32, tag="C2")
            Pt = singles.tile([64, NB], f32, tag="Pt")
            Qt = singles.tile([64, NB], f32, tag="Qt")
            nc.vector.memset(V, 0.0)
            nc.vector.memset(C2, 0.0)

            wA = {0: ws['w0A'], 1: ws['w1A'], 2: ws['w2A']}
            wB = {0: ws['w0B'], 1: ws['w1B'], 2: ws['w2B']}

            xtile = None
            for s in range(T + 2):
                ls = [l for l in (0, 1, 2) if 0 <= s - l < T]
                c0, c1 = min(ls) * 32, (max(ls) + 1) * 32
                cs = slice(c0, c1)

                if s % XCHUNK == 0 and s < T:
                    nch = min(XCHUNK, T - s)
                    xtile = xpool.tile([128, XCHUNK * BC], bf16, tag="xt")
                    nc.sync.dma_start(
                        out=xtile[:, 0:nch * BC], in_=xT_d[:, s * BC:(s + nch) * BC])

                pA = psum.tile([128, 2 * NB], f32, tag="pA")
                # bias init for all 192 cols (start=True clears the window)
                nc.tensor.matmul(pA, bvals, ind,
                                 start=True, stop=False, skip_group_check=True)
                if 0 in ls:
                    k = (s % XCHUNK) * BC
                    xs = xtile[:, k:k + BC]
                    nc.tensor.matmul(pA[:, 0:32], ws['wxA'], xs,
                                     start=False, stop=False, skip_group_check=True)
                    nc.tensor.matmul(pA[:, 96:128], ws['wxB'], xs,
                                     start=False, stop=False, skip_group_check=True)
                for l in ls:
                    cl = slice(32 * l, 32 * l + 32)
                    clB = slice(96 + 32 * l, 96 + 32 * l + 32)
                    nc.tensor.matmul(pA[:, cl], wA[l], V[:, cl],
                                     start=False, stop=True, skip_group_check=True)
                    nc.tensor.matmul(pA[:, clB], wB[l], V[:, cl],
                                     start=False, stop=True, skip_group_check=True)

                TT = scr.tile([128, 2 * NB], bf16, tag="TT")
                TC = scr.tile([64, NB], bf16, tag="TC")

                # one fused tanh over all gates of all layers
                nc.scalar.activation(TT, pA, AF.Tanh)
                # Q = (TF + 1) * C2
                nc.vector.scalar_tensor_tensor(
                    Qt, TT[0:64, 0:96], 1.0, C2,
                    ALU.add, ALU.mult)
                # P = (TI + 1) * TG   [partition-shifted write 64:128 -> 0:64]
                nc.vector.scalar_tensor_tensor(
                    Pt, TT[64:128, 0:96], 1.0, TT[64:128, 96:192],
                    ALU.add, ALU.mult)
                # C2 = (Q * 0.5) + P   (sliced: protects inactive layers' state)
                nc.vector.scalar_tensor_tensor(
                    C2[:, cs], Qt[:, cs], 0.5, Pt[:, cs],
                    ALU.mult, ALU.add)
                # TC = tanh(C2 * 0.5) = tanh(c)
                nc.scalar.activation(TC, C2, AF.Tanh, scale=0.5)
                # V state half: H2_l = (TO + 1) * TC   [shift 0:64 -> 64:128]
                nc.vector.scalar_tensor_tensor(
                    V[64:128, cs], TT[0:64, 96 + c0:96 + c1], 1.0, TC[:, cs],
                    ALU.add, ALU.mult)
                # V input half for layers l+1 (active l < 2)
                f0, f1 = c0, min(c1, 64)
                if f0 < f1:
                    nc.vector.scalar_tensor_tensor(
                        V[0:64, 32 + f0:32 + f1],
                        TT[0:64, 96 + f0:96 + f1], 1.0, TC[:, f0:f1],
                        ALU.add, ALU.mult)

            # final linear on layer-2 h(T-1):  V[64:128, 64:96] holds 2h
            H2f = singles.tile([64, BC], bf16, tag="H2f")
            nc.vector.tensor_copy(H2f, V[64:128, 64:96])
            po = psum_o.tile([2, BC], f32, tag="po")
            nc.tensor.matmul(po, wout, H2f, start=True, stop=True)
            outT = singles.tile([2, BC], f32, tag="outT")
            nc.scalar.copy(outT, po)
            nc.sync.dma_start(out=out_d[:, :], in_=outT)

    nc.compile()
    return nc


def pack_operands(W):
    wall = np.zeros((128, 8 * 128 + 2), BF16)
    for k, n in enumerate(['wxA', 'wxB', 'w0A', 'w0B', 'w1A', 'w1B', 'w2A', 'w2B']):
        wall[:, 128 * k:128 * (k + 1)] = W[n]
    wall[0:64, 1024:1026] = W['wout']
    fall = np.zeros((6, 128 + 192), BF16)
    fall[:, 0:128] = W['bvals']
    fall[:, 128:320] = W['ind']
    return wall, fall


def make_in_maps(inputs):
    W = _prep_weights(inputs)
    wall, fall = pack_operands(W)
    x = inputs['x'].astype(np.float32)
    in_maps = []
    for c in range(NCORES):
        xc = x[c * BC:(c + 1) * BC]                        # [BC, T, I]
        xT = np.ascontiguousarray(xc.transpose(2, 1, 0).reshape(I, T * BC)).astype(BF16)
        in_maps.append({'xT': xT, 'wall': wall, 'fall': fall})
    return in_maps


def kernel(**inputs):
    from concourse.bass_utils import run_bass_kernel_spmd

    if 'nc' not in _cache:
        _cache['nc'] = _build_program()
    nc = _cache['nc']

    in_maps = make_in_maps(inputs)
    res = run_bass_kernel_spmd(nc, in_maps, list(range(NCORES)))
    outs = [res.results[c]['out'].T for c in range(NCORES)]   # each [BC, 2]
    full = np.concatenate(outs, axis=0).astype(np.float32)
    full = full + inputs['b_out'].astype(np.float32)[None, :]
    return full


# revision 5
# speedup vs baseline: 60.2155x; 1.0377x over previous
"""3-layer LSTM (B=256, T=512, I=128, H=64) + final linear, on 8 TRN2 NeuronCores.

Strategy (data-parallel: batch 256 -> 32 per core; weights replicated):
  - Wavefront over the 3 layers: at step s, layer l computes timestep
    t = s - l, so each step advances all three layers with one set of
    wide fused instructions and the serial dependency chain is T+2 steps.
  - ALL-TANH gates: the i,f,o rows of weights/biases are pre-scaled by
    0.5 so sigmoid(x) = (tanh(x/2)+1)/2; g rows stay full scale.  One
    fused activation instruction computes tanh over the whole [128,192]
    PSUM gate tile = all 4 gates x 3 layers (partition layout: [f;i]
    for cols 0:96, [o;g] for cols 96:192), replacing two per-bank
    activations.
  - Cell update via fused scalar_tensor_tensor (DVE-only instruction):
        P  = (TI + 1) * TG          # = 2*i*g
        Q  = (TF + 1) * C2          # = 4*f*c   (state C2 = 2c)
        C2 = (Q * 0.5) + P          # = 2c_new
    then TC = tanh(C2 * 0.5) on the ACT engine (scale folds the /2),
    and the h update is fused directly into the V-state writes:
        V[64:128, :] = (TO + 1) * TC      # = 2h   (state half)
        V[0:64, 32:96] = (TO + 1) * TC    # forward feed to layer l+1
    so no separate U/H2 tiles or tensor copies exist on the chain.
  - One bf16 bias matmul (K=6 indicator) initializes all 192 psum
    columns per step (start=True), replacing two fp32 bias matmuls.
  - V [128, 96] bf16: partitions 0:64 hold the input half (H2_{l-1}),
    64:128 the state half (H2_l); recurrent matmuls are K=128 with
    M=128 bf16 stationaries (FWL-eligible), 6 per step.
  - x is transposed/cast on the host to [I, T*Bc] bf16 and streamed in
    16-step chunks (triple buffered, off the critical chain).
"""
import numpy as np
import ml_dtypes

B, T, I, H = 256, 512, 128, 64
NCORES = 8
BC = B // NCORES            # 32 batch per core
NB = 3 * BC                 # 96
XCHUNK = 16

BF16 = ml_dtypes.bfloat16
_cache = {}

# PyTorch gate row order: i(0:64) f(64:128) g(128:192) o(192:256).
_permA = np.r_[64:128, 0:64]       # [f; i]
_permB = np.r_[192:256, 128:192]   # [o; g]
_sA = np.full(128, 0.5, np.float32)              # f,i rows halved
_sB = np.r_[np.full(64, 0.5, np.float32),        # o rows halved
            np.full(64, 1.0, np.float32)]        # g rows full


def _prep_weights(inputs):
    f32 = np.float32
    W = {}
    for l in range(3):
        Wih = inputs[f'W_ih{l}'].astype(f32)
        Whh = inputs[f'W_hh{l}'].astype(f32)
        b = (inputs[f'b_ih{l}'] + inputs[f'b_hh{l}']).astype(f32)
        for perm, s, tag in ((_permA, _sA, 'A'), (_permB, _sB, 'B')):
            # lhsT[k, gate] layout, gate row scale s, h-consuming cols x0.5
            if l == 0:
                W[f'wx{tag}'] = (Wih[perm].T * s[None, :]).astype(BF16)
                m = np.zeros((128, 128), f32)
                m[64:128, :] = Whh[perm].T * s[None, :] * 0.5
                W[f'w0{tag}'] = m.astype(BF16)
            else:
                m = np.concatenate([Wih[perm].T, Whh[perm].T], axis=0)
                m = m * s[None, :] * 0.5
                W[f'w{l}{tag}'] = m.astype(BF16)
            W.setdefault(f'bias{tag}', []).append(b[perm] * s)
    # bvals [6, 128]: rows 0..2 = A-region bias per layer, 3..5 = B-region.
    W['bvals'] = np.stack(W.pop('biasA') + W.pop('biasB')).astype(BF16)
    ind = np.zeros((6, 192), f32)
    for l in range(3):
        ind[l, 32 * l:32 * l + 32] = 1.0
        ind[3 + l, 96 + 32 * l:96 + 32 * l + 32] = 1.0
    W['ind'] = ind.astype(BF16)
    W['wout'] = (inputs['W_out'].astype(f32).T * 0.5).astype(BF16)  # [64, 2]
    return W


def _build_program():
    import concourse.bacc as bacc
    import concourse.tile as tile
    from concourse import mybir

    AF = mybir.ActivationFunctionType
    ALU = mybir.AluOpType
    bf16 = mybir.dt.bfloat16
    f32 = mybir.dt.float32

    nc = bacc.Bacc(None, target_bir_lowering=False, debug=False)
    xT_d = nc.dram_tensor("xT", [128, T * BC], bf16, kind="ExternalInput")
    wnames = ['wxA', 'wxB', 'w0A', 'w0B', 'w1A', 'w1B', 'w2A', 'w2B']
    wall_d = nc.dram_tensor("wall", [128, 8 * 128 + 2], bf16, kind="ExternalInput")
    fall_d = nc.dram_tensor("fall", [6, 128 + 192], bf16, kind="ExternalInput")
    out_d = nc.dram_tensor("out", [2, BC], f32, kind="ExternalOutput")

    with tile.TileContext(nc) as tc:
        with (
            tc.tile_pool(name="singles", bufs=1) as singles,
            tc.tile_pool(name="xpool", bufs=3) as xpool,
            tc.tile_pool(name="scr", bufs=3) as scr,
            tc.tile_pool(name="psum", bufs=2, space="PSUM") as psum,
            tc.tile_pool(name="psum_o", bufs=1, space="PSUM") as psum_o,
        ):
            wall = singles.tile([128, 8 * 128 + 2], bf16, tag="wall")
            nc.sync.dma_start(out=wall, in_=wall_d[:, :])
            fall = singles.tile([6, 128 + 192], bf16, tag="fall")
            nc.sync.dma_start(out=fall, in_=fall_d[:, :])
            ws = {n: wall[:, 128 * k:128 * (k + 1)] for k, n in enumerate(wnames)}
            wout = wall[0:64, 8 * 128:8 * 128 + 2]
            bvals = fall[:, 0:128]
            ind = fall[:, 128:128 + 192]

            V0 = singles.tile([128, NB], bf16, tag="V0")
            V1 = singles.tile([128, NB], bf16, tag="V1")
            Vb = [V0, V1]
            C2 = singles.tile([64, NB], f32, tag="C2")
            Pt = singles.tile([64, NB], f32, tag="Pt")
            Qt = singles.tile([64, NB], f32, tag="Qt")
            nc.vector.memset(V0, 0.0)
            nc.vector.memset(V1, 0.0)
            nc.vector.memset(C2, 0.0)

            wA = {0: ws['w0A'], 1: ws['w1A'], 2: ws['w2A']}
            wB = {0: ws['w0B'], 1: ws['w1B'], 2: ws['w2B']}

            xtile = None
            for s in range(T + 4):
                ls = [l for l in (0, 1, 2) if 0 <= s - 2 * l < T]
                c0, c1 = min(ls) * 32, (max(ls) + 1) * 32
                cs = slice(c0, c1)
                V = Vb[s % 2]          # read buffer for this step's matmuls
                Vn = Vb[(s + 1) % 2]   # state written for step s+1

                if s % XCHUNK == 0 and s < T:
                    nch = min(XCHUNK, T - s)
                    xtile = xpool.tile([128, XCHUNK * BC], bf16, tag="xt")
                    nc.sync.dma_start(
                        out=xtile[:, 0:nch * BC], in_=xT_d[:, s * BC:(s + nch) * BC])

                pA = psum.tile([128, 2 * NB], f32, tag="pA")
                # bias init for all 192 cols (start=True clears the window)
                nc.tensor.matmul(pA, bvals, ind,
                                 start=True, stop=False, skip_group_check=True)
                if 0 in ls:
                    k = (s % XCHUNK) * BC
                    xs = xtile[:, k:k + BC]
                    nc.tensor.matmul(pA[:, 0:32], ws['wxA'], xs,
                                     start=False, stop=False, skip_group_check=True)
                    nc.tensor.matmul(pA[:, 96:128], ws['wxB'], xs,
                                     start=False, stop=False, skip_group_check=True)
                for l in ls:
                    cl = slice(32 * l, 32 * l + 32)
                    clB = slice(96 + 32 * l, 96 + 32 * l + 32)
                    nc.tensor.matmul(pA[:, cl], wA[l], V[:, cl],
                                     start=False, stop=True, skip_group_check=True)
                    nc.tensor.matmul(pA[:, clB], wB[l], V[:, cl],
                                     start=False, stop=True, skip_group_check=True)

                TT = scr.tile([128, 2 * NB], bf16, tag="TT")
                TC = scr.tile([64, NB], bf16, tag="TC")

                # one fused tanh over all gates of all layers
                nc.scalar.activation(TT, pA, AF.Tanh)
                # P = (TI + 1) * TG   [partition-shifted write 64:128 -> 0:64]
                nc.vector.scalar_tensor_tensor(
                    Pt, TT[64:128, 0:96], 1.0, TT[64:128, 96:192],
                    ALU.add, ALU.mult)
                # Q = (TF + 1) * C2
                nc.vector.scalar_tensor_tensor(
                    Qt, TT[0:64, 0:96], 1.0, C2,
                    ALU.add, ALU.mult)
                # C2 = (Q * 0.5) + P   (sliced: protects inactive layers' state)
                nc.vector.scalar_tensor_tensor(
                    C2[:, cs], Qt[:, cs], 0.5, Pt[:, cs],
                    ALU.mult, ALU.add)
                # TC = tanh(C2 * 0.5) = tanh(c)
                nc.scalar.activation(TC, C2, AF.Tanh, scale=0.5)
                # V state half: H2_l = (TO + 1) * TC   [shift 0:64 -> 64:128]
                # written into next step's buffer (consumed at s+1)
                nc.vector.scalar_tensor_tensor(
                    Vn[64:128, cs], TT[0:64, 96 + c0:96 + c1], 1.0, TC[:, cs],
                    ALU.add, ALU.mult)
                # V input half for layers l+1: consumed at s+2 (same parity),
                # so this write has a full step of slack -- off the chain.
                f0, f1 = c0, min(c1, 64)
                if f0 < f1:
                    nc.vector.scalar_tensor_tensor(
                        V[0:64, 32 + f0:32 + f1],
                        TT[0:64, 96 + f0:96 + f1], 1.0, TC[:, f0:f1],
                        ALU.add, ALU.mult)

            # final linear on layer-2 h(T-1): written at s=T+3 into Vb[(T+4)%2]
            H2f = singles.tile([64, BC], bf16, tag="H2f")
            nc.vector.tensor_copy(H2f, Vb[(T + 4) % 2][64:128, 64:96])
            po = psum_o.tile([2, BC], f32, tag="po")
            nc.tensor.matmul(po, wout, H2f, start=True, stop=True)
            outT = singles.tile([2, BC], f32, tag="outT")
            nc.scalar.copy(outT, po)
            nc.sync.dma_start(out=out_d[:, :], in_=outT)

    nc.compile()
    return nc


def pack_operands(W):
    wall = np.zeros((128, 8 * 128 + 2), BF16)
    for k, n in enumerate(['wxA', 'wxB', 'w0A', 'w0B', 'w1A', 'w1B', 'w2A', 'w2B']):
        wall[:, 128 * k:128 * (k + 1)] = W[n]
    wall[0:64, 1024:1026] = W['wout']
    fall = np.zeros((6, 128 + 192), BF16)
    fall[:, 0:128] = W['bvals']
    fall[:, 128:320] = W['ind']
    return wall, fall


def make_in_maps(inputs):
    W = _prep_weights(inputs)
    wall, fall = pack_operands(W)
    x = inputs['x'].astype(np.float32)
    in_maps = []
    for c in range(NCORES):
        xc = x[c * BC:(c + 1) * BC]                        # [BC, T, I]
        xT = np.ascontiguousarray(xc.transpose(2, 1, 0).reshape(I, T * BC)).astype(BF16)
        in_maps.append({'xT': xT, 'wall': wall, 'fall': fall})
    return in_maps


def kernel(**inputs):
    from concourse.bass_utils import run_bass_kernel_spmd

    inputs = {k: np.asarray(v) for k, v in inputs.items()}
    if 'nc' not in _cache:
        _cache['nc'] = _build_program()
    nc = _cache['nc']

    in_maps = make_in_maps(inputs)
    res = run_bass_kernel_spmd(nc, in_maps, list(range(NCORES)))
    outs = [res.results[c]['out'].T for c in range(NCORES)]   # each [BC, 2]
    full = np.concatenate(outs, axis=0).astype(np.float32)
    full = full + inputs['b_out'].astype(np.float32)[None, :]
    return full


# revision 8
# speedup vs baseline: 61.4652x; 1.0208x over previous
"""3-layer LSTM (B=256, T=512, I=128, H=64) + final linear, on 8 TRN2 NeuronCores.

Strategy (data-parallel: batch 256 -> 32 per core; weights replicated):
  - Skew-2 wavefront over the 3 layers: at step s, layer l computes
    timestep t = s - 2l (T+4 steps total).  The inter-layer offset of 2
    gives the layer->layer+1 forward-feed write (H2B) a full step of
    slack, taking it off the per-step critical chain: the recurrent
    matmuls wait only on the same-layer state write (H2A).  V is
    double-buffered by step parity to make that legal.
  - ALL-SIGMOID gates: one fused sigmoid over the whole [128,192] PSUM
    gate tile covers all 4 gates x 3 layers (partition layout: [f;i]
    for cols 0:96, [o;g] for 96:192).  f,i,o come out directly; the g
    rows are pre-doubled so g = 2*sigmoid(2*a_g) - 1 = tanh(a_g).
  - Cell update (stt = scalar_tensor_tensor, a DVE-only instruction):
        P = (Sg - 0.5) * Si         # = i*g/2   (stt)
        Q = Sf * C                  # = f*c     (tensor_tensor, 2x mode)
        C = (P * 2) + Q             # = c_new   (stt)
    then TC = tanh(C) on the ACT engine, and the h update is a plain
    multiply (2x-eligible) fused directly into the V writes:
        Vnext[64:128, :]  = So * TC   # = h  state half (on chain)
        Vcur[0:64, 32:96] = So * TC   # forward feed, consumed at s+2
                                      # (same parity, off chain)
    so no separate U/H2 tiles or tensor copies exist on the chain.
  - TT/TC are fp16 (2-byte keeps DVE 2x modes; 10 mantissa bits keep
    the gate quantization error low); V stays bf16 for the matmuls.
  - One bf16 bias matmul (K=6 indicator) initializes all 192 psum
    columns per step (start=True), replacing two fp32 bias matmuls.
  - V [128, 96] bf16: partitions 0:64 hold the input half (H2_{l-1}),
    64:128 the state half (H2_l); recurrent matmuls are K=128 with
    M=128 bf16 stationaries (FWL-eligible), 6 per step.
  - x is transposed/cast on the host to [I, T*Bc] bf16 and streamed in
    16-step chunks (triple buffered, off the critical chain).
"""
import numpy as np
import ml_dtypes

B, T, I, H = 256, 512, 128, 64
NCORES = 8
BC = B // NCORES            # 32 batch per core
NB = 3 * BC                 # 96
XCHUNK = 16

BF16 = ml_dtypes.bfloat16
_cache = {}

# PyTorch gate row order: i(0:64) f(64:128) g(128:192) o(192:256).
_permA = np.r_[64:128, 0:64]       # [f; i]
_permB = np.r_[192:256, 128:192]   # [o; g]
_sA = np.full(128, 1.0, np.float32)              # f,i rows (sigmoid direct)
_sB = np.r_[np.full(64, 1.0, np.float32),        # o rows (sigmoid direct)
            np.full(64, 2.0, np.float32)]        # g rows x2: g = 2*sig(2a)-1


def _prep_weights(inputs):
    f32 = np.float32
    W = {}
    for l in range(3):
        Wih = inputs[f'W_ih{l}'].astype(f32)
        Whh = inputs[f'W_hh{l}'].astype(f32)
        b = (inputs[f'b_ih{l}'] + inputs[f'b_hh{l}']).astype(f32)
        for perm, s, tag in ((_permA, _sA, 'A'), (_permB, _sB, 'B')):
            # lhsT[k, gate] layout, gate row scale s, h-consuming cols x0.5
            if l == 0:
                W[f'wx{tag}'] = (Wih[perm].T * s[None, :]).astype(BF16)
                m = np.zeros((128, 128), f32)
                m[64:128, :] = Whh[perm].T * s[None, :]
                W[f'w0{tag}'] = m.astype(BF16)
            else:
                m = np.concatenate([Wih[perm].T, Whh[perm].T], axis=0)
                m = m * s[None, :]
                W[f'w{l}{tag}'] = m.astype(BF16)
            W.setdefault(f'bias{tag}', []).append(b[perm] * s)
    # bvals [6, 128]: rows 0..2 = A-region bias per layer, 3..5 = B-region.
    W['bvals'] = np.stack(W.pop('biasA') + W.pop('biasB')).astype(BF16)
    ind = np.zeros((6, 192), f32)
    for l in range(3):
        ind[l, 32 * l:32 * l + 32] = 1.0
        ind[3 + l, 96 + 32 * l:96 + 32 * l + 32] = 1.0
    W['ind'] = ind.astype(BF16)
    W['wout'] = inputs['W_out'].astype(f32).T.astype(BF16)  # [64, 2]
    return W


def _build_program():
    import concourse.bacc as bacc
    import concourse.tile as tile
    from concourse import mybir

    AF = mybir.ActivationFunctionType
    ALU = mybir.AluOpType
    bf16 = mybir.dt.bfloat16
    fp16 = mybir.dt.float16
    f32 = mybir.dt.float32

    nc = bacc.Bacc(None, target_bir_lowering=False, debug=False)
    xT_d = nc.dram_tensor("xT", [128, T * BC], bf16, kind="ExternalInput")
    wnames = ['wxA', 'wxB', 'w0A', 'w0B', 'w1A', 'w1B', 'w2A', 'w2B']
    wall_d = nc.dram_tensor("wall", [128, 8 * 128 + 2], bf16, kind="ExternalInput")
    fall_d = nc.dram_tensor("fall", [6, 128 + 192], bf16, kind="ExternalInput")
    out_d = nc.dram_tensor("out", [2, BC], f32, kind="ExternalOutput")

    with tile.TileContext(nc) as tc:
        with (
            tc.tile_pool(name="singles", bufs=1) as singles,
            tc.tile_pool(name="xpool", bufs=3) as xpool,
            tc.tile_pool(name="scr", bufs=3) as scr,
            tc.tile_pool(name="psum", bufs=2, space="PSUM") as psum,
            tc.tile_pool(name="psum_o", bufs=1, space="PSUM") as psum_o,
        ):
            wall = singles.tile([128, 8 * 128 + 2], bf16, tag="wall")
            nc.sync.dma_start(out=wall, in_=wall_d[:, :])
            fall = singles.tile([6, 128 + 192], bf16, tag="fall")
            nc.sync.dma_start(out=fall, in_=fall_d[:, :])
            ws = {n: wall[:, 128 * k:128 * (k + 1)] for k, n in enumerate(wnames)}
            wout = wall[0:64, 8 * 128:8 * 128 + 2]
            bvals = fall[:, 0:128]
            ind = fall[:, 128:128 + 192]

            V0 = singles.tile([128, NB], bf16, tag="V0")
            V1 = singles.tile([128, NB], bf16, tag="V1")
            Vb = [V0, V1]
            C2 = singles.tile([64, NB], f32, tag="C2")
            Pt = singles.tile([64, NB], f32, tag="Pt")
            Qt = singles.tile([64, NB], f32, tag="Qt")
            nc.vector.memset(V0, 0.0)
            nc.vector.memset(V1, 0.0)
            nc.vector.memset(C2, 0.0)

            wA = {0: ws['w0A'], 1: ws['w1A'], 2: ws['w2A']}
            wB = {0: ws['w0B'], 1: ws['w1B'], 2: ws['w2B']}

            xtile = None
            for s in range(T + 4):
                ls = [l for l in (0, 1, 2) if 0 <= s - 2 * l < T]
                c0, c1 = min(ls) * 32, (max(ls) + 1) * 32
                cs = slice(c0, c1)
                V = Vb[s % 2]          # read buffer for this step's matmuls
                Vn = Vb[(s + 1) % 2]   # state written for step s+1

                if s % XCHUNK == 0 and s < T:
                    nch = min(XCHUNK, T - s)
                    xtile = xpool.tile([128, XCHUNK * BC], bf16, tag="xt")
                    nc.sync.dma_start(
                        out=xtile[:, 0:nch * BC], in_=xT_d[:, s * BC:(s + nch) * BC])

                pA = psum.tile([128, 2 * NB], f32, tag="pA")
                # bias init for all 192 cols (start=True clears the window)
                nc.tensor.matmul(pA, bvals, ind,
                                 start=True, stop=False, skip_group_check=True)
                if 0 in ls:
                    k = (s % XCHUNK) * BC
                    xs = xtile[:, k:k + BC]
                    nc.tensor.matmul(pA[:, 0:32], ws['wxA'], xs,
                                     start=False, stop=False, skip_group_check=True)
                    nc.tensor.matmul(pA[:, 96:128], ws['wxB'], xs,
                                     start=False, stop=False, skip_group_check=True)
                for l in ls:
                    cl = slice(32 * l, 32 * l + 32)
                    clB = slice(96 + 32 * l, 96 + 32 * l + 32)
                    nc.tensor.matmul(pA[:, cl], wA[l], V[:, cl],
                                     start=False, stop=True, skip_group_check=True)
                    nc.tensor.matmul(pA[:, clB], wB[l], V[:, cl],
                                     start=False, stop=True, skip_group_check=True)

                TT = scr.tile([128, 2 * NB], fp16, tag="TT")
                TC = scr.tile([64, NB], fp16, tag="TC")

                # one fused sigmoid over all gates of all layers
                # (f,i,o direct; g = 2*sig(2a)-1 via pre-doubled g rows)
                nc.scalar.activation(TT, pA, AF.Sigmoid)
                # P = (Sg - 0.5) * Si = i*g/2   [shifted write 64:128 -> 0:64]
                nc.vector.scalar_tensor_tensor(
                    Pt, TT[64:128, 96:192], 0.5, TT[64:128, 0:96],
                    ALU.subtract, ALU.mult)
                # Q = Sf * C = f*c   (plain tensor_tensor; state C = c)
                nc.vector.tensor_mul(Qt, TT[0:64, 0:96], C2)
                # C = (P * 2) + Q = c'  (sliced: protects inactive state)
                nc.vector.scalar_tensor_tensor(
                    C2[:, cs], Pt[:, cs], 2.0, Qt[:, cs],
                    ALU.mult, ALU.add)
                # TC = tanh(C) = tanh(c)
                nc.scalar.activation(TC, C2, AF.Tanh)
                # V state half: H2_l = (TO + 1) * TC   [shift 0:64 -> 64:128]
                # written into next step's buffer (consumed at s+1)
                nc.vector.tensor_mul(
                    Vn[64:128, cs], TT[0:64, 96 + c0:96 + c1], TC[:, cs])
                # V input half for layers l+1: consumed at s+2 (same parity),
                # so this write has a full step of slack -- off the chain.
                f0, f1 = c0, min(c1, 64)
                if f0 < f1:
                    nc.vector.tensor_mul(
                        V[0:64, 32 + f0:32 + f1],
                        TT[0:64, 96 + f0:96 + f1], TC[:, f0:f1])

            # final linear on layer-2 h(T-1): written at s=T+3 into Vb[(T+4)%2]
            H2f = singles.tile([64, BC], bf16, tag="H2f")
            nc.vector.tensor_copy(H2f, Vb[(T + 4) % 2][64:128, 64:96])
            po = psum_o.tile([2, BC], f32, tag="po")
            nc.tensor.matmul(po, wout, H2f, start=True, stop=True)
            outT = singles.tile([2, BC], f32, tag="outT")
            nc.scalar.copy(outT, po)
            nc.sync.dma_start(out=out_d[:, :], in_=outT)

    nc.compile()
    return nc


def pack_operands(W):
    wall = np.zeros((128, 8 * 128 + 2), BF16)
    for k, n in enumerate(['wxA', 'wxB', 'w0A', 'w0B', 'w1A', 'w1B', 'w2A', 'w2B']):
        wall[:, 128 * k:128 * (k + 1)] = W[n]
    wall[0:64, 1024:1026] = W['wout']
    fall = np.zeros((6, 128 + 192), BF16)
    fall[:, 0:128] = W['bvals']
    fall[:, 128:320] = W['ind']
    return wall, fall


def make_in_maps(inputs):
    W = _prep_weights(inputs)
    wall, fall = pack_operands(W)
    x = inputs['x'].astype(np.float32)
    in_maps = []
    for c in range(NCORES):
        xc = x[c * BC:(c + 1) * BC]                        # [BC, T, I]
        xT = np.ascontiguousarray(xc.transpose(2, 1, 0).reshape(I, T * BC)).astype(BF16)
        in_maps.append({'xT': xT, 'wall': wall, 'fall': fall})
    return in_maps


def kernel(**inputs):
    from concourse.bass_utils import run_bass_kernel_spmd

    inputs = {k: np.asarray(v) for k, v in inputs.items()}
    if 'nc' not in _cache:
        _cache['nc'] = _build_program()
    nc = _cache['nc']

    in_maps = make_in_maps(inputs)
    res = run_bass_kernel_spmd(nc, in_maps, list(range(NCORES)))
    outs = [res.results[c]['out'].T for c in range(NCORES)]   # each [BC, 2]
    full = np.concatenate(outs, axis=0).astype(np.float32)
    full = full + inputs['b_out'].astype(np.float32)[None, :]
    return full
